# revision 32
# baseline (speedup 1.0000x reference)
"""GAT layer (4 heads, N=4096, E=131072) as a Trainium2 Bass/Tile SPMD kernel.

Row-partitioned: core d owns destination rows [d*512, (d+1)*512). Host
preprocessing is index-only plus dtype marshalling (bf16 hi/lo split of
node_feats). Per-core node indices are rotated (pi(n) = (n - d*512) % N)
so each core's own rows are local blocks 0..3 — all per-core selection
becomes compile-time slicing.

Device phase A (replicated): s-pass first (s = nf @ A2 from bf16 hi/lo
node-feature pairs, f32-accurate logit scalars; s2 to DRAM hes [N, 64]
f32 rows, p-major: row = p*32 + nb); then h = nf @ W.T on PE with
(f,h)-interleaved output columns (col f*4+h), cast to bf16 and written
to DRAM heh [N, 128] rows.

Phase B: edges bucketed per (tile, 64-row window), sorted by dst. Two
f32 dma_gathers per 1024-edge call (elem 256B = the HW minimum; 4-byte
gather dtype — 8-byte gather dtypes silently corrupt on HW): s2 by dst
from hes, h rows by dst from heh. The src-side term s1 is NOT gathered:
per chunk, a K=64 matmul broadcasts the window's 64 s1 values to edge
slots via the transposed one-hot map mre (K=64 because matmul operand
slices below K=64 are unreliable on HW). logit = s1 + s2 on DVE (psum +
sbuf), leaky-relu on DVE, exp batched on the scalar engine; u = h*v on
DVE where the (f,h)-interleaved layout makes the per-head broadcast
stride-1-last (2x DVE mode); the last group's u is split per call so it
trails each gather. Aggregation via per-chunk 128x(128+4) matmuls into
64-row psum bands of a [128, 512] psum tile per row-tile (start/stop
flags, no memset; gpsimd never touches PSUM); normalize by reciprocal;
one 128-row output DMA per tile.

Only the two gathers run on the Pool engine (gathers serialize on it,
costed at bytes/614GBps); every other DMA is issued from SP/Act/DVE
queues.

Known limit: a destination row with zero in-edges would produce NaN
(the dense reference gives a uniform mean); impossible here (min
degree 14).
"""

import numpy as np
import ml_dtypes

import concourse.bass as bass
import concourse.bacc as bacc
import concourse.mybir as mybir
import concourse.tile as tile
from concourse import bass_utils

F32 = mybir.dt.float32
BF16 = mybir.dt.bfloat16
I16 = mybir.dt.int16
I64 = mybir.dt.int64

N = 4096
CIN = 128
H = 4
CH = 32
FEAT = H * CH  # 128
NCORES = 8
RPC = N // NCORES  # 512 rows per core
RT = RPC // 128    # 4 row-tiles per core
NB = N // 128      # 32 node blocks
NW = 2             # 64-row windows per tile
WROWS = 128 // NW  # 64 rows per window
ALPHA = 0.2
GCALL = 1024
CPC = GCALL // 128  # 8 chunks per gather call
GR = 3              # gather calls per phase-B group

_BUILD_CACHE: dict[tuple, object] = {}


def _build(TQ: int, has_bias: bool):
    """Per-core program; TQ = chunks (of 128 edge slots) per 64-row window."""
    T = NW * TQ          # chunks per 128-row tile
    TC = RT * TQ         # mre columns (t, cw)
    C = RT * T           # chunks per core
    L = C * 128          # edge slots per core
    NCALL = L // GCALL   # gather calls
    NG = -(-NCALL // GR)  # phase-B groups (last may be ragged)

    nc = bacc.Bacc("TRN2", target_bir_lowering=False, debug=False,
                   enable_asserts=False, num_devices=NCORES)

    # ---- I/O ----
    nfh_in = nc.dram_tensor("nfh", [CIN, N], BF16, kind="ExternalInput").ap()
    nfl_in = nc.dram_tensor("nfl", [CIN, N], BF16, kind="ExternalInput").ap()
    w_in = nc.dram_tensor("w", [FEAT, CIN], F32, kind="ExternalInput").ap()
    wtp_in = nc.dram_tensor("wtp", [CIN, FEAT], F32, kind="ExternalInput").ap()
    brp_in = nc.dram_tensor("brp", [1, FEAT], F32, kind="ExternalInput").ap()
    bcol_in = nc.dram_tensor("bcol", [FEAT, 1], F32, kind="ExternalInput").ap()
    acat_in = nc.dram_tensor("acat", [FEAT, 8], F32, kind="ExternalInput").ap()
    mt_in = nc.dram_tensor("mt", [128, C, WROWS], BF16, kind="ExternalInput").ap()
    mre_in = nc.dram_tensor("mre", [128, TC, 128], BF16, kind="ExternalInput").ap()
    gidx_in = nc.dram_tensor("gidx", [128, L // 16], I16, kind="ExternalInput").ap()
    out_d = nc.dram_tensor("out", [RPC, FEAT], F32, kind="ExternalOutput").ap()

    with tile.TileContext(nc) as tc:
        with (
            tc.tile_pool(name="const", bufs=1) as cp,
            tc.tile_pool(name="dram", bufs=1, space="DRAM") as dp,
            tc.tile_pool(name="work", bufs=2) as wp,
            tc.tile_pool(name="psO", bufs=1, space="PSUM") as psO,
            tc.tile_pool(name="psL", bufs=1, space="PSUM") as psL,
        ):
            psA_cm = tc.tile_pool(name="psA", bufs=2, space="PSUM")
            psA = psA_cm.__enter__()
            psS_cm = tc.tile_pool(name="psS", bufs=1, space="PSUM")
            psS = psS_cm.__enter__()

            # ---- loads: w/acat first (A2 gates the s-pass), then nf
            # quarters split SP/Act; wtp+gidx ride the idle Pool queue ----
            w_sb = cp.tile([FEAT, CIN], F32)
            nc.sync.dma_start(out=w_sb[:], in_=w_in)
            acat_sb = cp.tile([FEAT, 8], F32)
            nc.sync.dma_start(out=acat_sb[:], in_=acat_in)
            wtp_sb = cp.tile([CIN, FEAT], F32)
            nc.gpsimd.dma_start(out=wtp_sb[:], in_=wtp_in)
            gidx_sb = cp.tile([128, L // 16], I16)
            nc.gpsimd.dma_start(out=gidx_sb[:], in_=gidx_in)
            wtp16_sb = cp.tile([CIN, FEAT], BF16)
            nc.vector.tensor_copy(out=wtp16_sb[:], in_=wtp_sb[:])

            NQ = N // 4
            nfh_sb = cp.tile([CIN, N], BF16)
            nfl_sb = cp.tile([CIN, N], BF16)
            for q4 in range(4):
                qs = slice(q4 * NQ, (q4 + 1) * NQ)
                eng = nc.sync if q4 % 2 == 0 else nc.scalar
                eng.dma_start(out=nfh_sb[:, qs], in_=nfh_in[:, qs])
            for q4 in range(4):
                qs = slice(q4 * NQ, (q4 + 1) * NQ)
                eng = nc.sync if q4 % 2 == 0 else nc.scalar
                eng.dma_start(out=nfl_sb[:, qs], in_=nfl_in[:, qs])

            if has_bias:
                brp_sb = cp.tile([1, FEAT], F32)
                nc.sync.dma_start(out=brp_sb[:], in_=brp_in)
                bcol_sb = cp.tile([FEAT, 1], F32)
                nc.sync.dma_start(out=bcol_sb[:], in_=bcol_in)
                brp16_sb = cp.tile([1, FEAT], BF16)
                nc.vector.tensor_copy(out=brp16_sb[:], in_=brp_sb[:])
                ones16_sb = cp.tile([1, FEAT], BF16)
                nc.vector.memset(ones16_sb[:], 1.0)
                ones_sb = cp.tile([1, FEAT], F32)
                nc.vector.memset(ones_sb[:], 1.0)

            # A2 = W.T @ a_cat (f32), then hi/lo bf16 split
            ps_a2 = psS.tile([CIN, 8], F32, tag="psA2")
            nc.tensor.matmul(ps_a2[:], lhsT=w_sb[:], rhs=acat_sb[:],
                             start=True, stop=True)
            A2h_sb = cp.tile([CIN, 8], BF16)
            nc.vector.tensor_copy(out=A2h_sb[:], in_=ps_a2[:])
            A2l_sb = cp.tile([CIN, 8], BF16)
            nc.vector.tensor_tensor(out=A2l_sb[:], in0=ps_a2[:], in1=A2h_sb[:],
                                    op=mybir.AluOpType.subtract)
            if has_bias:
                ps_sb = psS.tile([1, 8], F32, tag="psA2")
                nc.tensor.matmul(ps_sb[:], lhsT=bcol_sb[:], rhs=acat_sb[:],
                                 start=True, stop=True)
                sbias_sb = cp.tile([1, 8], F32)
                nc.vector.tensor_copy(out=sbias_sb[:], in_=ps_sb[:])

            # ---- DRAM scratch: h rows (256B) + s2 rows, p-major ----
            heh = dp.tile([N, FEAT], BF16)
            hes = dp.tile([N, 64], F32)
            s2f_sb = cp.tile([128, NB, 4], F32)

            # ---- phase A: s-pass first (gathers gate on its hes rows) ----
            ps_s = psS.tile([128, NB, 8], F32, tag="psS")
            for nb in range(NB):
                lh = nfh_sb[:, nb * 128:(nb + 1) * 128]
                ll = nfl_sb[:, nb * 128:(nb + 1) * 128]
                nc.tensor.matmul(ps_s[:, nb, :], lhsT=lh, rhs=A2h_sb[:],
                                 start=True, stop=False)
                nc.tensor.matmul(ps_s[:, nb, :], lhsT=lh, rhs=A2l_sb[:],
                                 start=False, stop=False)
                nc.tensor.matmul(ps_s[:, nb, :], lhsT=ll, rhs=A2h_sb[:],
                                 start=False, stop=not has_bias)
                if has_bias:
                    nc.tensor.matmul(ps_s[:, nb, :], lhsT=ones_sb[:],
                                     rhs=sbias_sb[:], start=False, stop=True)
            nc.vector.tensor_copy(out=s2f_sb[:], in_=ps_s[:, :, 4:8])
            # s1 for own rows, bf16, partition-aligned with mre blocks:
            # s1w[q*64+r, t, h] = s1 of local row t*128 + q*64 + r
            s1w_sb = cp.tile([128, RT, 4], BF16)
            nc.vector.tensor_copy(out=s1w_sb[:], in_=ps_s[:, 0:RT, 0:4])
            hes_v = hes[:].rearrange("(p nb) f -> p nb f", nb=NB)
            nc.gpsimd.dma_start(out=hes_v[:, :, 0:4], in_=s2f_sb[:])

            # mre first (gates the s1-broadcast matmuls): Act takes the
            # early half (its queue is shorter), SP the late half
            mre_sb = cp.tile([128, TC, 128], BF16)
            nc.scalar.dma_start(out=mre_sb[:, 0:TC // 2, :],
                                in_=mre_in[:, 0:TC // 2, :])
            nc.sync.dma_start(out=mre_sb[:, TC // 2:, :],
                              in_=mre_in[:, TC // 2:, :])

            # ---- phase A: h-pass ----
            heh_v = heh[:].rearrange("(p nb) f -> p nb f", nb=NB)
            h16_eng = [2, 1, 2, 1, 2, 1, 2, 1]  # 1=scalar 2=vector
            for g in range(NB // 4):
                ps_h = psA.tile([128, 4, FEAT], F32, tag="psA")
                for qb in range(4):
                    nb = g * 4 + qb
                    lh = nfh_sb[:, nb * 128:(nb + 1) * 128]
                    nc.tensor.matmul(ps_h[:, qb, :], lhsT=lh, rhs=wtp16_sb[:],
                                     start=True, stop=not has_bias)
                    if has_bias:
                        nc.tensor.matmul(ps_h[:, qb, :], lhsT=ones16_sb[:],
                                         rhs=brp16_sb[:], start=False,
                                         stop=True)
                bs = slice(g * 4, (g + 1) * 4)
                h16 = wp.tile([128, 4, FEAT], BF16, tag="h16", bufs=8)
                if h16_eng[g] == 1:
                    nc.scalar.copy(out=h16[:], in_=ps_h[:])
                else:
                    nc.vector.tensor_copy(out=h16[:], in_=ps_h[:])
                eng = nc.sync if g % 2 == 0 else nc.scalar
                eng.dma_start(out=heh_v[:, bs, :], in_=h16[:])

            psS_cm.__exit__(None, None, None)
            psA_cm.__exit__(None, None, None)

            # mt halves after heh stores
            mt_sb = cp.tile([128, C, WROWS], BF16)
            nc.sync.dma_start(out=mt_sb[:, 0:C // 2, :],
                              in_=mt_in[:, 0:C // 2, :])
            nc.scalar.dma_start(out=mt_sb[:, C // 2:, :],
                                in_=mt_in[:, C // 2:, :])

            # ---- phase B: all s1-broadcast matmuls first (PE executes in
            # order; these must not interleave with the aggregation's open
            # psum accumulation groups) ----
            heh32 = heh[:].bitcast(F32)   # [N, 64]
            hes32 = hes[:]                # [N, 64] f32
            assert NCALL % GR == 0, NCALL
            KPG = GR * CPC
            # split the per-slot s1 psum at a group boundary (2 banks)
            CH2 = ((NG + 1) // 2) * KPG
            ps_l0 = psL.tile([128, CH2, 4], F32, tag="psl0")
            ps_l1 = psL.tile([128, C - CH2, 4], F32, tag="psl1")
            ps_ls = (ps_l0, ps_l1)
            for k in range(C):
                t, ct = divmod(k, T)
                q, cw = divmod(ct, TQ)
                qs = slice(q * WROWS, (q + 1) * WROWS)
                nc.tensor.matmul(ps_ls[k // CH2][:, k % CH2, :],
                                 lhsT=mre_sb[qs, t * TQ + cw, :],
                                 rhs=s1w_sb[qs, t, :],
                                 start=True, stop=True)

            u_tiles = [cp.tile([128, KPG, FEAT + 4], BF16, tag=f"u{gi}",
                               name=f"ut{gi}") for gi in range(NG)]
            gs_tiles = [wp.tile([128, KPG, 64], F32, tag="gs", bufs=3,
                                name=f"gst{gi}") for gi in range(NG)]
            gh_tiles = [wp.tile([128, KPG, 64], F32, tag="gh", bufs=3,
                                name=f"ght{gi}") for gi in range(NG)]
            v16_tiles = [wp.tile([128, KPG, 4], BF16, tag=f"v16{gi}",
                                 name=f"v16t{gi}") for gi in range(NG)]

            def gather_calls(gi, dst_tile, src):
                for jj in range(GR):
                    j = gi * GR + jj
                    nc.gpsimd.dma_gather(
                        out_ap=dst_tile[:, jj * CPC:(jj + 1) * CPC, :],
                        in_ap=src,
                        idxs_ap=gidx_sb[:, j * 64:(j + 1) * 64],
                        num_idxs=GCALL, num_idxs_reg=GCALL, elem_size=64)

            def logit_chain(gi):
                k0 = gi * KPG
                logit = wp.tile([128, KPG, 4], F32, tag="logit", bufs=3)
                pl = ps_ls[k0 // CH2]
                nc.vector.tensor_tensor(out=logit[:],
                                        in0=pl[:, k0 % CH2:k0 % CH2 + KPG, :],
                                        in1=gs_tiles[gi][:, :, 0:4],
                                        op=mybir.AluOpType.add)
                nc.vector.scalar_tensor_tensor(
                    out=logit[:], in0=logit[:], scalar=ALPHA, in1=logit[:],
                    op0=mybir.AluOpType.mult, op1=mybir.AluOpType.max)
                nc.scalar.activation(out=v16_tiles[gi][:], in_=logit[:],
                                     func=mybir.ActivationFunctionType.Exp)

            # pool order: per group, s2 gathers then h gathers
            for gi in range(NG):
                gather_calls(gi, gs_tiles[gi], hes32)
                logit_chain(gi)
                gather_calls(gi, gh_tiles[gi], heh32)

            # ---- u = h*v, with tile normalize/store interleaved into the
            # DVE queue as soon as the covering groups are done ----
            o_sb = cp.tile([128, RT, FEAT], F32)
            out_v = out_d.rearrange("(t p) f -> p t f", t=RT)
            # agg psum per tile; 2 rotating banks, allocated at first use
            ps_os = {}

            def ps_o_tile(t):
                if t not in ps_os:
                    ps_os[t] = psO.tile([128, 512], F32, tag=f"psO{t % 2}",
                                        name=f"pso{t}")
                return ps_os[t]

            def normalize(t):
                ps_o = ps_os[t]
                rec_sb = wp.tile([128, 4], F32, tag="rec", bufs=3)
                nc.vector.reciprocal(out=rec_sb[:],
                                     in_=ps_o[:, FEAT:FEAT + 4])
                # (f,h)-interleaved -> standard (h,f) output order
                nc.vector.tensor_tensor(
                    out=o_sb[:, t, :].rearrange("p (h f) -> p h f", h=H),
                    in0=ps_o[:, 0:FEAT].rearrange("p (f h) -> p h f", h=H),
                    in1=rec_sb[:, :, None].to_broadcast([128, H, CH]),
                    op=mybir.AluOpType.mult)
                eng = nc.sync if t % 2 == 0 else nc.scalar
                ts = slice(t, t + 1)
                eng.dma_start(out=out_v[:, ts, :], in_=o_sb[:, ts, :])

            # group-major chunk order equals global (tile-major) chunk order,
            # so aggregation is emitted per group right after its u tile, and
            # each tile's normalize right after the group that completes it —
            # this places normalizes early in the DVE queue instead of behind
            # every u-mult.
            done_g = [((t + 1) * T - 1) // KPG for t in range(RT)]
            for gi in range(NG):
                ghv = gh_tiles[gi][:].bitcast(BF16)  # [128, KPG, 128]
                v16 = v16_tiles[gi]
                ut = u_tiles[gi]
                # last group: per-call pieces so the final u trails each
                # gather instead of waiting for the whole triplet
                pieces = GR if gi == NG - 1 else 1
                w = KPG // pieces
                for pc in range(pieces):
                    cs = slice(pc * w, (pc + 1) * w)
                    nc.vector.tensor_tensor(
                        out=ut[:, cs, 0:FEAT].rearrange(
                            "p c (f h) -> p c f h", h=H),
                        in0=ghv[:, cs, :].rearrange(
                            "p c (f h) -> p c f h", h=H),
                        in1=v16[:, cs, None, :].to_broadcast(
                            [128, w, CH, H]),
                        op=mybir.AluOpType.mult)
                    nc.scalar.copy(out=ut[:, cs, FEAT:FEAT + 4],
                                   in_=v16[:, cs, :])
                # aggregation matmuls for this group's chunks
                # (window q of tile t -> partitions q*64:(q+1)*64)
                for k in range(gi * KPG, (gi + 1) * KPG):
                    t, ct = divmod(k, T)
                    q, cw = divmod(ct, TQ)
                    rl = WROWS * q
                    nc.tensor.matmul(ps_o_tile(t)[rl:rl + WROWS, 0:FEAT + 4],
                                     lhsT=mt_sb[:, k, :],
                                     rhs=ut[:, k - gi * KPG, :],
                                     start=(cw == 0), stop=(cw == TQ - 1),
                                     skip_group_check=True)
                for t in range(RT):
                    if done_g[t] == gi:
                        normalize(t)

    nc.compile()
    return nc


def _get_build(TQ: int, has_bias: bool):
    key = (TQ, has_bias)
    if key not in _BUILD_CACHE:
        _BUILD_CACHE[key] = _build(TQ, has_bias)
    return _BUILD_CACHE[key]


def _wrap_gather_idx(idx: np.ndarray, L: int) -> np.ndarray:
    """Pack index list (len L, multiple of 1024) into the [128, L/16] int16
    layout dma_gather wants."""
    out = np.zeros((128, L // 16), np.int16)
    for j in range(L // 1024):
        blk = idx[j * 1024:(j + 1) * 1024].astype(np.int16).reshape(64, 16).T
        for c in range(8):
            out[16 * c:16 * (c + 1), j * 64:(j + 1) * 64] = blk
    return out


def kernel(**inputs) -> np.ndarray:
    node_feats = np.asarray(inputs["node_feats"], dtype=np.float32)
    W = np.asarray(inputs["W"], dtype=np.float32)
    b = np.asarray(inputs["b"], dtype=np.float32)
    a = np.asarray(inputs["a"], dtype=np.float32)
    edge_index = np.asarray(inputs["edge_index"])

    src = edge_index[0].astype(np.int64)
    dst = edge_index[1].astype(np.int64)
    # dedup (matches dense .at[].set semantics; duplicate logits identical)
    keys = np.unique(src * N + dst)
    su = (keys // N).astype(np.int64)
    du = (keys % N).astype(np.int64)

    # per-core rotated indices; bucket by (tile, 64-row window), sort by dst
    own = su // RPC               # owning core
    psrc_all = su - own * RPC     # local row 0..511
    tile_all = psrc_all // 128
    win_all = (psrc_all % 128) // WROWS
    bucket = ((own * RT + tile_all) * NW + win_all)
    order = np.lexsort((du, bucket))
    su, du, bucket = su[order], du[order], bucket[order]
    own, psrc_all = own[order], psrc_all[order]
    counts = np.bincount(bucket, minlength=NCORES * RT * NW)
    starts = np.zeros(NCORES * RT * NW + 1, np.int64)
    np.cumsum(counts, out=starts[1:])
    TQ = int(-(-counts.max() // 128))
    # round call count (NCALL == TQ here) up to a multiple of GR so phase-B
    # groups are uniform
    TQ = -(-TQ // GR) * GR
    T = NW * TQ
    TC = RT * TQ
    C = RT * T
    L = C * 128

    nc = _get_build(TQ, bool(np.any(b)))

    # constant marshalling
    a_cat = np.zeros((FEAT, 8), np.float32)
    for hh in range(H):
        a_cat[hh * CH:(hh + 1) * CH, hh] = a[hh, :CH]
        a_cat[hh * CH:(hh + 1) * CH, 4 + hh] = a[hh, CH:]
    # (f,h)-interleaved projection columns: new col f*4+h = old col h*32+f
    perm = np.arange(FEAT).reshape(H, CH).T.reshape(-1)  # perm[f*4+h]=h*32+f
    Wt_perm = np.ascontiguousarray(W.T[:, perm])
    brp = b[perm].reshape(1, FEAT)
    bcol = b.reshape(FEAT, 1)
    nf_T = node_feats.T                      # [CIN, N]
    nfh_T = nf_T.astype(ml_dtypes.bfloat16)  # hi/lo dtype marshalling
    nfl_T = (nf_T - nfh_T.astype(np.float32)).astype(ml_dtypes.bfloat16)

    jj64 = np.arange(WROWS)
    in_maps = []
    for d in range(NCORES):
        # pi rotation: own nodes first
        nfh_d = np.ascontiguousarray(np.roll(nfh_T, -d * RPC, axis=1))
        nfl_d = np.ascontiguousarray(np.roll(nfl_T, -d * RPC, axis=1))
        gidx = np.zeros(L, np.int64)
        rloc = np.full((128, C), -1.0, np.float32)  # row-in-window per slot
        for t in range(RT):
            for q in range(NW):
                bid = (d * RT + t) * NW + q
                lo, n_e = starts[bid], counts[bid]
                cs = (t * NW + q) * TQ * 128       # first slot of window
                rel = np.full(TQ * 128, -1.0, np.float32)
                rel[:n_e] = (psrc_all[lo:lo + n_e] % WROWS).astype(np.float32)
                rloc[:, cs // 128:(cs // 128) + TQ] = rel.reshape(TQ, 128).T
                pd = (du[lo:lo + n_e] - d * RPC) % N   # rotated dst
                gi = np.zeros(TQ * 128, np.int64)
                gi[:n_e] = (pd % 128) * NB + pd // 128  # p-major heh row
                gidx[cs:cs + TQ * 128] = gi
        mt = (rloc[:, :, None] == jj64[None, None, :]).astype(ml_dtypes.bfloat16)
        # mre [128 (q*64+r), TC (t*TQ+cw), 128 e]
        mre = np.zeros((128, TC, 128), ml_dtypes.bfloat16)
        mt4 = mt.reshape(128, RT, NW, TQ, WROWS)   # [e, t, q, cw, r]
        for q in range(NW):
            mre[WROWS * q:WROWS * (q + 1)] = (
                mt4[:, :, q, :, :].transpose(3, 1, 2, 0).reshape(WROWS, TC, 128))
        in_maps.append({
            "nfh": nfh_d, "nfl": nfl_d, "w": W, "wtp": Wt_perm,
            "brp": brp, "bcol": bcol, "acat": a_cat,
            "mt": np.ascontiguousarray(mt), "mre": np.ascontiguousarray(mre),
            "gidx": _wrap_gather_idx(gidx, L),
        })

    res = None
    for attempt in range(3):
        try:
            res = bass_utils.run_bass_kernel_spmd(nc, in_maps,
                                                  core_ids=list(range(NCORES)))
            break
        except Exception:
            if attempt == 2:
                raise
    out = np.concatenate([res.results[d]["out"] for d in range(NCORES)],
                         axis=0)
    return np.ascontiguousarray(out.astype(np.float32))
